# revision 23
# baseline (speedup 1.0000x reference)
"""Chamfer distance kernel for Trainium2 — v5 (single sweep + DMA transpose).

One matmul sweep serves BOTH directions:
- d[n, m] strips (128-row blocks x 8192 cols) are ACT-copied to bf16.
- Direction 1 (row minima): DVE bf16 fold tree (2x) to 256 positions per
  block; position s covers the 32 columns congruent to s (mod 256).
- Direction 2 (column minima): each 2048-col strip is DMA-TRANSPOSED
  (xbar, on the otherwise idle SP queue; trans[p,q,r] = strip[r, q*128+p])
  and min-accumulated across the 64 row blocks with bf16 tensor_tensor
  at 2x. The final acc holds, per column, the min over rows of each
  row-slot r (row = 128*b + r) -> host refines the 64 rows of the argmin
  slot exactly in f32.
Host refinement gives exact idx AND exact dist for every row/col (bf16
rounding is monotone; ties are unioned).
"""

import numpy as np
import ml_dtypes

import concourse.bacc as bacc
import concourse.mybir as mybir
from concourse import tile
from concourse.bass_utils import run_bass_kernel_spmd

F32 = mybir.dt.float32
BF16 = mybir.dt.bfloat16
AF = mybir.ActivationFunctionType
ALU = mybir.AluOpType

BF = ml_dtypes.bfloat16

_PROGRAM_CACHE = {}


def _build_program(n_pts=8192, n_cores=8, repeat=1):
    key = (n_pts, n_cores, repeat)
    if key in _PROGRAM_CACHE:
        return _PROGRAM_CACHE[key]

    NB = n_pts // 128          # 64 row blocks
    TW = 2048                  # strip width
    NS = n_pts // TW           # 4 strips
    NPOS = 256                 # dir-1 positions per block

    nc = bacc.Bacc("TRN2", target_bir_lowering=False, debug=False,
                   num_devices=n_cores)
    uu = nc.dram_tensor("uu", [96, n_pts], BF16, kind="ExternalInput")
    r1o = nc.dram_tensor("r1", [128, NB * NPOS], BF16, kind="ExternalOutput")
    r2o = nc.dram_tensor("r2", [128, n_pts], BF16, kind="ExternalOutput")

    with tile.TileContext(nc) as tc:
        with tc.tile_pool(name="persist", bufs=1) as persist:
            U1 = persist.tile([56, n_pts], BF16, tag="U1")
            U2 = persist.tile([56, n_pts], BF16, tag="U2")
            racc1 = persist.tile([128, NB * NPOS], BF16, tag="racc1")

            nc.sync.dma_start(U1[0:24, :], uu.ap()[0:24, :])
            nc.sync.dma_start(U1[32:56, :], uu.ap()[24:48, :])
            nc.sync.dma_start(U2[0:24, :], uu.ap()[72:96, :])
            nc.sync.dma_start(U2[32:56, :], uu.ap()[48:72, :])

            with tc.tile_pool(name="psum", bufs=4, space="PSUM") as pspool, \
                 tc.tile_pool(name="tt", bufs=2) as tpool, \
                 tc.tile_pool(name="tr", bufs=6) as trpool, \
                 tc.tile_pool(name="ac", bufs=2) as accpool, \
                 tc.tile_pool(name="vv", bufs=2) as vpool, \
                 tc.tile_pool(name="ww", bufs=2) as wpool, \
                 tc.tile_pool(name="xx", bufs=3) as xpool, \
                 tc.tile_pool(name="yy", bufs=3) as ypool:
                for _ in range(repeat):
                    acc_prev = None
                    for nb in range(NB):
                        lhs = U1[0:24, nb * 128:(nb + 1) * 128]
                        T = tpool.tile([128, n_pts], BF16, tag="T")
                        for k in range(2 * NS):
                            ps = pspool.tile([128, 1024], F32, tag="ps")
                            for q in range(2):
                                c0 = k * 1024 + q * 512
                                nc.tensor.matmul(
                                    ps[:, q * 512:(q + 1) * 512],
                                    lhs,
                                    U2[0:24, c0:c0 + 512],
                                    start=True, stop=True)
                            nc.scalar.activation(
                                T[:, k * 1024:(k + 1) * 1024],
                                ps[:], AF.Copy)
                        # dir-1: bf16 fold tree (2x) to NPOS positions
                        V0 = vpool.tile([128, TW], BF16, tag="V0")
                        nc.vector.tensor_tensor(
                            V0[:], T[:, 0:TW], T[:, TW:2 * TW], ALU.min)
                        V1 = vpool.tile([128, TW], BF16, tag="V1")
                        nc.vector.tensor_tensor(
                            V1[:], T[:, 2 * TW:3 * TW], T[:, 3 * TW:4 * TW],
                            ALU.min)
                        W = wpool.tile([128, TW], BF16, tag="W")
                        nc.vector.tensor_tensor(W[:], V0[:], V1[:], ALU.min)
                        X = xpool.tile([128, 1024], BF16, tag="X")
                        nc.vector.tensor_tensor(
                            X[:], W[:, 0:1024], W[:, 1024:2048], ALU.min)
                        Y = ypool.tile([128, 512], BF16, tag="Y")
                        nc.vector.tensor_tensor(
                            Y[:], X[:, 0:512], X[:, 512:1024], ALU.min)
                        nc.vector.tensor_tensor(
                            racc1[:, nb * NPOS:(nb + 1) * NPOS],
                            Y[:, 0:256], Y[:, 256:512], ALU.min)
                        # dir-2: xbar transpose each strip (SP queue), then
                        # min-accumulate across blocks (bf16 TT at 2x)
                        acc = accpool.tile([128, n_pts], BF16, tag="acc")
                        for st in range(NS):
                            tr = trpool.tile([128, 16, 128], BF16, tag="tr")
                            nc.sync.dma_start_transpose(
                                tr[:], T[:, st * TW:(st + 1) * TW])
                            trf = tr[:].rearrange("p q r -> p (q r)")
                            prev = (trf if acc_prev is None
                                    else acc_prev[:, st * TW:(st + 1) * TW])
                            nc.vector.tensor_tensor(
                                acc[:, st * TW:(st + 1) * TW],
                                prev, trf, ALU.min)
                        acc_prev = acc

                    nc.sync.dma_start(r2o.ap(), acc_prev[:])
            nc.sync.dma_start(r1o.ap(), racc1[:])

    nc.compile()
    _PROGRAM_CACHE[key] = nc
    return nc


def _split3(v):
    h = v.astype(BF).astype(np.float32)
    r = (v - h).astype(np.float32)
    m = r.astype(BF).astype(np.float32)
    l = (r - m).astype(BF).astype(np.float32)
    return h, m, l


def _forms(xyz):
    """[N,3] f32 -> (A, B) [24, N] bf16 triple-split homogeneous forms."""
    x = np.ascontiguousarray(xyz.T).astype(np.float32)
    n = (x * x).sum(0, dtype=np.float32)[None, :]
    s = (-2.0 * x).astype(np.float32)
    sh, sm, sl = _split3(s)
    xh, xm, xl = _split3(x)
    nh, nm, nl = _split3(n)
    ones = np.ones_like(n)
    A = np.concatenate([sh, sh, sm, sh, sl, sm, ones, ones, ones,
                        nh, nm, nl]).astype(BF)
    Bf = np.concatenate([xh, xm, xh, xl, xh, xm, nh, nm, nl,
                         ones, ones, ones]).astype(BF)
    return A, Bf


def _fold_maps(n_pts=8192):
    """dir-1: position s covers the 32 columns congruent to s (mod 256)."""
    cols = np.arange(n_pts)
    s = cols % 256
    cands = np.empty((256, 32), np.int64)
    for p in range(256):
        cc = np.nonzero(s == p)[0]
        assert cc.size == 32
        cands[p] = cc
    return s, cands


_COLMAP, _CANDS = _fold_maps()


def _refine(R, Xq, Xc):
    """dir-1 refine: R [128, NB*256] bf16 position minima.
    Exact f32 recompute of the <=32 candidate cols per row."""
    npos = _CANDS.shape[0]
    NB = R.shape[1] // npos
    N = NB * 128
    vals = np.asarray(R).reshape(128, NB, npos) \
        .transpose(1, 0, 2).reshape(N, npos)
    vf = vals.astype(np.float32)
    m = vf.min(1)
    smin = vf.argmin(1)
    nties = (vf == m[:, None]).sum(1)
    cands = _CANDS[smin]
    nc2 = (Xc * Xc).sum(1)
    nq = (Xq * Xq).sum(1)
    cpts = Xc[cands]
    d = nq[:, None] + nc2[cands] \
        - 2.0 * np.einsum('nd,nkd->nk', Xq, cpts)
    d = np.maximum(d.astype(np.float32), 0.0)
    loc = d.argmin(1)
    idx = cands[np.arange(N), loc]
    dist = d[np.arange(N), loc]
    rows = np.nonzero(nties > 1)[0]
    for r in rows:
        ss = np.nonzero(vf[r] == m[r])[0]
        cc = np.sort(np.concatenate([_CANDS[s] for s in ss]))
        dd = nq[r] + nc2[cc] - 2.0 * (Xc[cc] @ Xq[r])
        dd = np.maximum(dd.astype(np.float32), 0.0)
        l = dd.argmin()
        idx[r] = cc[l]
        dist[r] = dd[l]
    return dist, idx.astype(np.int32)


def _refine_cols(acc, Xq, Xc):
    """dir-2 refine: acc [128, 8192] bf16; acc[p, st*2048+q*128+r] = min
    over blocks b of d[128*b + r, col st*2048 + q*128 + p].
    Xq = xyz2 (cols, the queries), Xc = xyz1 (rows, the candidates)."""
    M = acc.shape[1]
    NBR = Xc.shape[0] // 128
    A = np.asarray(acc).reshape(128, M // 2048, 16, 128)
    # V[c, r]: c = st*2048 + q*128 + p
    V = A.transpose(1, 2, 0, 3).reshape(M, 128).astype(np.float32)
    m = V.min(1)
    rmin = V.argmin(1)
    nties = (V == m[:, None]).sum(1)
    cands = rmin[:, None] + 128 * np.arange(NBR)[None, :]      # [M, NB]
    nc2 = (Xc * Xc).sum(1)
    nq = (Xq * Xq).sum(1)
    cpts = Xc[cands]
    d = nq[:, None] + nc2[cands] \
        - 2.0 * np.einsum('nd,nkd->nk', Xq, cpts)
    d = np.maximum(d.astype(np.float32), 0.0)
    loc = d.argmin(1)
    idx = cands[np.arange(M), loc]
    dist = d[np.arange(M), loc]
    rows = np.nonzero(nties > 1)[0]
    for c in rows:
        rr = np.nonzero(V[c] == m[c])[0]
        cc = np.sort((rr[:, None] + 128 * np.arange(NBR)[None, :]).ravel())
        dd = nq[c] + nc2[cc] - 2.0 * (Xc[cc] @ Xq[c])
        dd = np.maximum(dd.astype(np.float32), 0.0)
        l = dd.argmin()
        idx[c] = cc[l]
        dist[c] = dd[l]
    return dist, idx.astype(np.int32)


def kernel(xyz1: np.ndarray, xyz2: np.ndarray, repeat: int = 1):
    xyz1 = np.asarray(xyz1, dtype=np.float32)
    xyz2 = np.asarray(xyz2, dtype=np.float32)
    B, N, _ = xyz1.shape
    M = xyz2.shape[1]
    assert B == 8 and N == 8192 and M == 8192, (B, N, M)

    nc = _build_program(N, B, repeat)

    in_maps = []
    for b in range(B):
        A1, B1 = _forms(xyz1[b])
        A2, B2 = _forms(xyz2[b])
        in_maps.append({"uu": np.concatenate([A1, B1, A2, B2])})
    res = run_bass_kernel_spmd(nc, in_maps, list(range(B)))

    dist1 = np.empty((B, N), np.float32)
    dist2 = np.empty((B, M), np.float32)
    idx1 = np.empty((B, N), np.int32)
    idx2 = np.empty((B, M), np.int32)
    for b in range(B):
        r = res.results[b]
        dist1[b], idx1[b] = _refine(np.asarray(r["r1"]), xyz1[b], xyz2[b])
        dist2[b], idx2[b] = _refine_cols(np.asarray(r["r2"]),
                                         xyz2[b], xyz1[b])
    return dist1, dist2, idx1, idx2
